# revision 2
# baseline (speedup 1.0000x reference)
"""Trainium2 Bass kernel for a LeakyReLU RNN — fp8-DoubleRow recurrence.

Model (B=128, S=512, I=256, H=1024, O=256):
    xproj = lrelu(x @ Wi.T + bi)                          # [B,S,H]
    h_t   = lrelu(concat(xproj_t, h_{t-1}) @ Wh.T + bh)   # recurrence over S
    out   = h_S @ Wo.T + bo                               # [B,O]

v2 strategy (vs v1 f32r baseline): data-parallel over batch (16 rows/core).
Phase 1 (U = lrelu(xWi+bi) @ Wh1.T + bh) stays f32r but produces U
pre-scaled by S_U = S_W*S_H (weights pre-scaled on host). Phase 2 runs the
recurrence h_t = lrelu(U_t + h_{t-1} @ Wh2.T) with Wh2 and h quantized to
fp8e4m3 and the 8 k-chunk matmuls fused into 4 DoubleRow matmuls
(virtual K=256, 2 fp8 pairs/cell), halving the dominant weight-stream
time. Scales: W2 *= S_W, h *= S_H (folded into the activation's
scale=1/S_W since lrelu is positively homogeneous), Wo /= S_H.
"""

from contextlib import ExitStack

import numpy as np
import ml_dtypes

import concourse.bacc as bacc
import concourse.tile as tile
from concourse import mybir
from concourse.bass_utils import run_bass_kernel_spmd

B, S, I, H, O = 128, 512, 256, 1024, 256
NCORES = 8
BL = B // NCORES          # batch rows per core = 16
TOK = BL * S              # tokens per core = 8192
NBLK = TOK // 512         # 512-token blocks in phase 1 = 16
RING_STEPS = 8            # recurrence steps per U ring DMA
ALPHA = 0.01

S_W = 4096.0              # Wh2 fp8 scale
S_H = 128.0               # h fp8 scale
S_U = S_W * S_H           # U pre-scale (2**19)

F32 = mybir.dt.float32
F32R = mybir.dt.float32r
F8 = mybir.dt.float8e4
BF16 = mybir.dt.bfloat16
LRELU = mybir.ActivationFunctionType.Lrelu
DR = mybir.MatmulPerfMode.DoubleRow

_CACHED = None


def _build(S=S, NBLK=NBLK):
    TOK = BL * S
    nc = bacc.Bacc("TRN2", target_bir_lowering=False, debug=False,
                   num_devices=NCORES)

    xt_d = nc.dram_tensor("xt", [I, TOK], F32, kind="ExternalInput")
    wit_d = nc.dram_tensor("wit", [I, H], F32, kind="ExternalInput")
    wh1t_d = nc.dram_tensor("wh1t", [H, H], F32, kind="ExternalInput")
    # DoubleRow-packed Wh2.T: [c*128+p, e, j] = S_W * Wh2[j, 256c+128e+p]
    wh2dr_d = nc.dram_tensor("wh2dr", [512, 2, H], F8, kind="ExternalInput")
    wot_d = nc.dram_tensor("wot", [H, O], F32, kind="ExternalInput")
    bi_d = nc.dram_tensor("bi", [128, H // 128], F32, kind="ExternalInput")
    bh_d = nc.dram_tensor("bh", [1, H], F32, kind="ExternalInput")
    bo_d = nc.dram_tensor("bo", [1, O], F32, kind="ExternalInput")
    eyeb_d = nc.dram_tensor("eyeb", [16, 16], BF16, kind="ExternalInput")
    ones_d = nc.dram_tensor("ones", [1, 512], F32, kind="ExternalInput")
    y_d = nc.dram_tensor("y", [BL, O], F32, kind="ExternalOutput")
    u_dram = nc.dram_tensor("udram", [128, 8, TOK], F32R)

    with tile.TileContext(nc) as tc, ExitStack() as ctx:
        wpool = ctx.enter_context(tc.tile_pool(name="weights", bufs=1))
        xtpool = ctx.enter_context(tc.tile_pool(name="xt", bufs=3))
        apool = ctx.enter_context(tc.tile_pool(name="atiles", bufs=2))
        upool = ctx.enter_context(tc.tile_pool(name="usb", bufs=4))
        ringpool = ctx.enter_context(tc.tile_pool(name="uring", bufs=4))
        hpool = ctx.enter_context(tc.tile_pool(name="hbuf", bufs=2))
        opool = ctx.enter_context(tc.tile_pool(name="osb", bufs=1))
        ps1ctx = ExitStack()
        psA = ps1ctx.enter_context(tc.tile_pool(name="psA", bufs=2, space="PSUM"))
        psU = ps1ctx.enter_context(tc.tile_pool(name="psU", bufs=4, space="PSUM"))

        # ---- resident weights ----
        def wload(src, shape, tag, dt=F32R):
            t = wpool.tile(shape, dt, tag=tag, name=tag)
            nc.gpsimd.dma_start(t[:], src)
            return t

        wit = [wload(wit_d.ap()[128 * k:128 * (k + 1), :], [128, H], f"wit{k}")
               for k in range(2)]
        wh1t = [wload(wh1t_d.ap()[128 * k:128 * (k + 1), :], [128, H], f"wh1t{k}")
                for k in range(8)]
        wdr = [wload(wh2dr_d.ap()[128 * c:128 * (c + 1), :, :], [128, 2, H],
                     f"wdr{c}", dt=F8) for c in range(4)]
        wot = [wload(wot_d.ap()[128 * k:128 * (k + 1), :], [128, O], f"wot{k}")
               for k in range(8)]
        eyeb = wload(eyeb_d.ap(), [16, 16], "eyeb", dt=BF16)
        bh2 = wload(bh_d.ap(), [1, H], "bh2")
        bo2 = wload(bo_d.ap(), [1, O], "bo2")
        ones = wload(ones_d.ap(), [1, 512], "ones")
        bi = wpool.tile([128, H // 128], F32, tag="bi", name="bi")
        nc.sync.dma_start(bi[:], bi_d.ap())

        # ---- phase 1: A_T = lrelu(WiT.T @ Xt + bi); U = S_U*(A @ Wh1.T + bh)
        # (wh1t and bh are pre-scaled by S_U on the host)
        for blk in range(NBLK):
            c0 = 512 * blk
            xt = [xtpool.tile([128, 512], F32R, tag=f"xt{k}", name=f"xt{k}_{blk}") for k in range(2)]
            for k in range(2):
                nc.gpsimd.dma_start(
                    xt[k][:], xt_d.ap()[128 * k:128 * (k + 1), c0:c0 + 512])
            a = []
            for m in range(8):
                pa = psA.tile([128, 512], F32, tag="psA", name=f"psA_{blk}_{m}")
                nc.tensor.matmul(pa[:], wit[0][:, 128 * m:128 * (m + 1)],
                                 xt[0][:], start=True, stop=False)
                nc.tensor.matmul(pa[:], wit[1][:, 128 * m:128 * (m + 1)],
                                 xt[1][:], start=False, stop=True)
                am = apool.tile([128, 512], F32R, tag=f"a{m}", name=f"a{m}_{blk}")
                nc.scalar.activation(am[:], pa[:], LRELU,
                                     bias=bi[:, m:m + 1], scale=1.0, alpha=ALPHA)
                a.append(am)
            for m in range(8):
                pum = psU.tile([128, 512], F32, tag="psU", name=f"psU_{blk}_{m}")
                nc.tensor.matmul(pum[:], bh2[0:1, 128 * m:128 * (m + 1)],
                                 ones[0:1, :], start=True, stop=False)
                for k in range(8):
                    nc.tensor.matmul(
                        pum[:], wh1t[k][:, 128 * m:128 * (m + 1)],
                        a[k][:], start=False, stop=(k == 7))
                usb = upool.tile([128, 512], F32R, tag="usb", name=f"usb_{blk}_{m}")
                nc.vector.tensor_copy(usb[:], pum[:])
                nc.scalar.dma_start(
                    u_dram.ap()[:, m:m + 1, c0:c0 + 512], usb[:])

        # ---- phase 2: recurrence (fp8 DoubleRow) ----
        ps1ctx.close()
        psR = ctx.enter_context(tc.tile_pool(name="psR", bufs=3, space="PSUM"))
        psT = ctx.enter_context(tc.tile_pool(name="psT", bufs=2, space="PSUM"))
        # hdr chunks: 4 tiles [128, 2, 16] fp8; (p, e, b) = S_H*h[b, 256c+128e+p]
        hdr = []
        for c in range(4):
            t = hpool.tile([128, 2, 16], F8, tag=f"hdr{c}", name=f"hdr{c}_init")
            nc.gpsimd.memset(t[:].bitcast(F32), 0.0)
            hdr.append(t)

        INV_SW = 1.0 / S_W
        COPYF = mybir.ActivationFunctionType.Copy

        def load_ring(r):
            ring_new = ringpool.tile([128, 8, RING_STEPS * BL], F32R, tag="ring",
                                     name=f"ring_{r}")
            nc.sync.dma_start(
                ring_new[:],
                u_dram.ap()[:, :, r * RING_STEPS * BL:(r + 1) * RING_STEPS * BL])
            return ring_new

        hf = None
        ring = load_ring(0)
        ring_nxt = load_ring(1)
        for t in range(S):
            g = t % RING_STEPS
            ps0 = psR.tile([16, 512], F32, tag="psR", name=f"psR0_{t}")
            ps1 = psR.tile([16, 512], F32, tag="psR", name=f"psR1_{t}")
            for c in range(4):
                nc.tensor.matmul(ps0[:], hdr[c][:, :, :], wdr[c][:, :, 0:512],
                                 start=(c == 0), stop=(c == 3), perf_mode=DR)
            for c in range(4):
                nc.tensor.matmul(ps1[:], hdr[c][:, :, :], wdr[c][:, :, 512:1024],
                                 start=(c == 0), stop=(c == 3), perf_mode=DR)
            if g == 0 and t > 0:
                ring = ring_nxt
                if t + RING_STEPS < S:
                    ring_nxt = load_ring(t // RING_STEPS + 1)
            # evac psum*(1/S_W) -> bf16, split across Scalar (w0) / DVE (w1)
            w0 = hpool.tile([16, 512], BF16, tag="w0", name=f"w0_{t}")
            nc.scalar.activation(w0[:], ps0[:], COPYF, bias=0.0, scale=INV_SW)
            w1 = hpool.tile([16, 512], BF16, tag="w1", name=f"w1_{t}")
            nc.vector.tensor_scalar_mul(w1[:], ps1[:], INV_SW)
            w = (w0, w1)
            last = (t == S - 1)
            hdr_new = []
            if last:
                hf = []
            for c in range(4):
                pt = psT.tile([128, 32], BF16, tag=f"pt{c}", name=f"pt{c}_{t}",
                              bufs=1)
                for e in range(2):
                    col = 256 * c + 128 * e
                    nc.tensor.transpose(pt[:, 16 * e:16 * (e + 1)],
                                        w[col // 512][:, col % 512:col % 512 + 128],
                                        eyeb[0:16, 0:16])
                # s = S_H*(h@Wh2.T)^T + S_H*U^T  (chunk c covers j-chunks 2c,2c+1)
                sc = hpool.tile([128, 2, 16], BF16, tag=f"s{c}", name=f"s{c}_{t}")
                nc.vector.tensor_add(sc[:, :, :], pt[:],
                                     ring[:, 2 * c:2 * c + 2,
                                          BL * g:BL * (g + 1)])
                ht = hpool.tile([128, 2, 16], F8, tag=f"hdr{c}", name=f"hdr{c}_{t}")
                nc.scalar.activation(ht[:, :, :], sc[:, :, :], LRELU,
                                     bias=0.0, scale=1.0, alpha=ALPHA)
                hdr_new.append(ht)
                if last:
                    hfc = hpool.tile([128, 2, 16], F32R, tag=f"hf{c}", name=f"hf{c}")
                    nc.scalar.activation(hfc[:, :, :], sc[:, :, :], LRELU,
                                         bias=0.0, scale=1.0, alpha=ALPHA)
                    hf.append(hfc)
            hdr = hdr_new

        # ---- phase 3: out = h_S @ (Wo.T/S_H) + bo  (hf holds S_H*h_S) ----
        po = psR.tile([16, O], F32, tag="psO", name="psO", bufs=1)
        nc.tensor.matmul(po[:], ones[0:1, 0:16], bo2[0:1, :],
                         start=True, stop=False)
        for k in range(8):
            c, e = k // 2, k % 2
            nc.tensor.matmul(po[:], hf[c][:, e, :], wot[k][:],
                             start=False, stop=(k == 7))
        osb = opool.tile([16, O], F32, tag="osb", name="osb")
        nc.vector.tensor_copy(osb[:], po[:])
        nc.sync.dma_start(y_d.ap(), osb[:])

    nc.compile()
    return nc


def _prep_inputs(x, Wi, bi, Wh, bh, Wo, bo):
    W2 = Wh[:, H:]                                  # [H(j), H(k)]
    w2t = np.ascontiguousarray(W2.T) * S_W          # [k, j]
    # [c, e, p, j] -> [c, p, e, j] -> [512, 2, H]
    wdr = w2t.reshape(4, 2, 128, H).transpose(0, 2, 1, 3).reshape(512, 2, H)
    shared = {
        "wit": np.ascontiguousarray(Wi.T),
        "wh1t": np.ascontiguousarray(Wh[:, :H].T) * S_H,
        "wh2dr": np.ascontiguousarray(wdr).astype(ml_dtypes.float8_e4m3),
        "wot": np.ascontiguousarray(Wo.T) / S_H,
        "bi": np.ascontiguousarray(bi.reshape(H // 128, 128).T),
        "bh": bh.reshape(1, H).copy() * S_H,
        "bo": bo.reshape(1, O).copy(),
        "eyeb": np.eye(16, dtype=np.float32).astype(ml_dtypes.bfloat16),
        "ones": np.ones((1, 512), np.float32),
    }
    in_maps = []
    for c in range(NCORES):
        xc = x[BL * c:BL * (c + 1)]            # [16, S, I]
        xt = np.ascontiguousarray(
            xc.transpose(2, 1, 0).reshape(I, TOK))  # [I, S*16] col = t*16+b
        m = dict(shared)
        m["xt"] = xt
        in_maps.append(m)
    return in_maps


def kernel(x, Wi, bi, Wh, bh, Wo, bo, _trace=False):
    global _CACHED
    x = np.asarray(x, dtype=np.float32)
    if _CACHED is None:
        _CACHED = _build()
    nc = _CACHED
    in_maps = _prep_inputs(np.asarray(x, np.float32), np.asarray(Wi, np.float32),
                           np.asarray(bi, np.float32), np.asarray(Wh, np.float32),
                           np.asarray(bh, np.float32), np.asarray(Wo, np.float32),
                           np.asarray(bo, np.float32))
    res = run_bass_kernel_spmd(nc, in_maps, list(range(NCORES)), trace=_trace)
    out = np.concatenate([res.results[c]["y"] for c in range(NCORES)], axis=0)
    if _trace:
        return out, res
    return out


# revision 3
# speedup vs baseline: 1.0000x; 1.0000x over previous
"""Trainium2 Bass kernel for a LeakyReLU RNN — fp8-DoubleRow recurrence.

Model (B=128, S=512, I=256, H=1024, O=256):
    xproj = lrelu(x @ Wi.T + bi)                          # [B,S,H]
    h_t   = lrelu(concat(xproj_t, h_{t-1}) @ Wh.T + bh)   # recurrence over S
    out   = h_S @ Wo.T + bo                               # [B,O]

v2 strategy (vs v1 f32r baseline): data-parallel over batch (16 rows/core).
Phase 1 (U = lrelu(xWi+bi) @ Wh1.T + bh) stays f32r but produces U
pre-scaled by S_U = S_W*S_H (weights pre-scaled on host). Phase 2 runs the
recurrence h_t = lrelu(U_t + h_{t-1} @ Wh2.T) with Wh2 and h quantized to
fp8e4m3 and the 8 k-chunk matmuls fused into 4 DoubleRow matmuls
(virtual K=256, 2 fp8 pairs/cell), halving the dominant weight-stream
time. Scales: W2 *= S_W, h *= S_H (folded into the activation's
scale=1/S_W since lrelu is positively homogeneous), Wo /= S_H.
"""

from contextlib import ExitStack

import numpy as np
import ml_dtypes

import concourse.bacc as bacc
import concourse.tile as tile
from concourse import mybir
from concourse.bass_utils import run_bass_kernel_spmd

B, S, I, H, O = 128, 512, 256, 1024, 256
NCORES = 8
BL = B // NCORES          # batch rows per core = 16
TOK = BL * S              # tokens per core = 8192
NBLK = TOK // 512         # 512-token blocks in phase 1 = 16
RING_STEPS = 8            # recurrence steps per U ring DMA
ALPHA = 0.01

S_W = 4096.0              # Wh2 fp8 scale
S_H = 128.0               # h fp8 scale
S_U = S_W * S_H           # U pre-scale (2**19)

F32 = mybir.dt.float32
F32R = mybir.dt.float32r
F8 = mybir.dt.float8e4
BF16 = mybir.dt.bfloat16
LRELU = mybir.ActivationFunctionType.Lrelu
DR = mybir.MatmulPerfMode.DoubleRow

_CACHED = None


def _build(S=S, NBLK=NBLK):
    TOK = BL * S
    nc = bacc.Bacc("TRN2", target_bir_lowering=False, debug=False,
                   num_devices=NCORES)

    xt_d = nc.dram_tensor("xt", [I, TOK], F32, kind="ExternalInput")
    wit_d = nc.dram_tensor("wit", [I, H], F32, kind="ExternalInput")
    wh1t_d = nc.dram_tensor("wh1t", [H, H], F32, kind="ExternalInput")
    # fp8 Wh2.T: [k, j] = S_W * Wh2[j, k]
    wh2f8_d = nc.dram_tensor("wh2f8", [H, H], F8, kind="ExternalInput")
    wot_d = nc.dram_tensor("wot", [H, O], F32, kind="ExternalInput")
    bi_d = nc.dram_tensor("bi", [128, H // 128], F32, kind="ExternalInput")
    bh_d = nc.dram_tensor("bh", [1, H], F32, kind="ExternalInput")
    bo_d = nc.dram_tensor("bo", [1, O], F32, kind="ExternalInput")
    eyeb_d = nc.dram_tensor("eyeb", [128, 16], BF16, kind="ExternalInput")
    ones_d = nc.dram_tensor("ones", [1, 512], F32, kind="ExternalInput")
    y_d = nc.dram_tensor("y", [BL, O], F32, kind="ExternalOutput")
    u_dram = nc.dram_tensor("udram", [128, 8, TOK], F32R)

    with tile.TileContext(nc) as tc, ExitStack() as ctx:
        wpool = ctx.enter_context(tc.tile_pool(name="weights", bufs=1))
        xtpool = ctx.enter_context(tc.tile_pool(name="xt", bufs=3))
        apool = ctx.enter_context(tc.tile_pool(name="atiles", bufs=2))
        upool = ctx.enter_context(tc.tile_pool(name="usb", bufs=4))
        ringpool = ctx.enter_context(tc.tile_pool(name="uring", bufs=4))
        hpool = ctx.enter_context(tc.tile_pool(name="hbuf", bufs=2))
        opool = ctx.enter_context(tc.tile_pool(name="osb", bufs=1))
        psR = ctx.enter_context(tc.tile_pool(name="psR", bufs=3, space="PSUM"))
        psT = ctx.enter_context(tc.tile_pool(name="psT", bufs=2, space="PSUM"))

        # ---- resident weights ----
        def wload(src, shape, tag, dt=F32R):
            t = wpool.tile(shape, dt, tag=tag, name=tag)
            nc.gpsimd.dma_start(t[:], src)
            return t

        wit = [wload(wit_d.ap()[128 * k:128 * (k + 1), :], [128, H], f"wit{k}")
               for k in range(2)]
        wh1t = [wload(wh1t_d.ap()[128 * k:128 * (k + 1), :], [128, H], f"wh1t{k}")
                for k in range(8)]
        w8 = [wload(wh2f8_d.ap()[128 * k:128 * (k + 1), :], [128, H],
                    f"w8_{k}", dt=F8) for k in range(8)]
        wot = [wload(wot_d.ap()[128 * k:128 * (k + 1), :], [128, O], f"wot{k}")
               for k in range(8)]
        eyeb = wload(eyeb_d.ap(), [128, 16], "eyeb", dt=BF16)
        bh2 = wload(bh_d.ap(), [1, H], "bh2")
        bo2 = wload(bo_d.ap(), [1, O], "bo2")
        ones = wload(ones_d.ap(), [1, 512], "ones")
        bi = wpool.tile([128, H // 128], F32, tag="bi", name="bi")
        nc.sync.dma_start(bi[:], bi_d.ap())

        # ---- phase 1: A_T = lrelu(WiT.T @ Xt + bi); U = S_U*(A @ Wh1.T + bh)
        # (wh1t and bh are pre-scaled by S_U on the host); emitted in blocks,
        # interleaved into the recurrence loop so its matmuls fill PE stalls.
        def emit_block(blk):
            c0 = 512 * blk
            xt = [xtpool.tile([128, 512], F32R, tag=f"xt{k}", name=f"xt{k}_{blk}") for k in range(2)]
            for k in range(2):
                nc.gpsimd.dma_start(
                    xt[k][:], xt_d.ap()[128 * k:128 * (k + 1), c0:c0 + 512])
            a = []
            for m in range(8):
                pa = psR.tile([128, 512], F32, tag="psR", name=f"psA_{blk}_{m}")
                nc.tensor.matmul(pa[:], wit[0][:, 128 * m:128 * (m + 1)],
                                 xt[0][:], start=True, stop=False)
                nc.tensor.matmul(pa[:], wit[1][:, 128 * m:128 * (m + 1)],
                                 xt[1][:], start=False, stop=True)
                am = apool.tile([128, 512], F32R, tag=f"a{m}", name=f"a{m}_{blk}")
                nc.scalar.activation(am[:], pa[:], LRELU,
                                     bias=bi[:, m:m + 1], scale=1.0, alpha=ALPHA)
                a.append(am)
            for m in range(8):
                pum = psR.tile([128, 512], F32, tag="psR", name=f"psU_{blk}_{m}")
                nc.tensor.matmul(pum[:], bh2[0:1, 128 * m:128 * (m + 1)],
                                 ones[0:1, :], start=True, stop=False)
                for k in range(8):
                    nc.tensor.matmul(
                        pum[:], wh1t[k][:, 128 * m:128 * (m + 1)],
                        a[k][:], start=False, stop=(k == 7))
                usb = upool.tile([128, 512], F32R, tag="usb", name=f"usb_{blk}_{m}")
                nc.vector.tensor_copy(usb[:], pum[:])
                nc.scalar.dma_start(
                    u_dram.ap()[:, m:m + 1, c0:c0 + 512], usb[:])

        # ---- phase 2: recurrence (fp8 col-tiled) ----
        # hdr chunks: 4 tiles [128, 2, 16] fp8; (p, e, b) = S_H*h[b, 256c+128e+p]
        hdr = []
        for c in range(4):
            t = hpool.tile([128, 2, 16], F8, tag=f"hdr{c}", name=f"hdr{c}_init")
            nc.gpsimd.memset(t[:].bitcast(F32), 0.0)
            hdr.append(t)

        INV_SW = 1.0 / S_W
        COPYF = mybir.ActivationFunctionType.Copy

        def load_ring(r):
            ring_new = ringpool.tile([128, 8, RING_STEPS * BL], F32R, tag="ring",
                                     name=f"ring_{r}")
            nc.sync.dma_start(
                ring_new[:],
                u_dram.ap()[:, :, r * RING_STEPS * BL:(r + 1) * RING_STEPS * BL])
            return ring_new

        hf = None
        emit_block(0)
        emit_block(1)
        ring = load_ring(0)
        ring_nxt = load_ring(1)
        for t in range(S):
            if t % 32 == 0 and t // 32 + 2 < NBLK:
                emit_block(t // 32 + 2)
            g = t % RING_STEPS
            ps = psR.tile([128, 512], F32, tag="psR", name=f"psR_{t}")
            # 4-way col-tiled: quarter q (j in [256q, 256q+256)) accumulates at
            # psum partitions [32q, 32q+16); one start=True clears the bank.
            for k in range(8):
                stat = hdr[k // 2][:, k % 2, :]
                for q in range(4):
                    nc.tensor.matmul(ps[32 * q:32 * q + 16, 0:256],
                                     stat, w8[k][:, 256 * q:256 * (q + 1)],
                                     start=(k == 0),
                                     stop=(k == 7),
                                     skip_group_check=True,
                                     tile_position=(0, 32 * q))
            if g == 0 and t > 0:
                ring = ring_nxt
                if t + RING_STEPS < S:
                    ring_nxt = load_ring(t // RING_STEPS + 1)
            # evac psum*(1/S_W) -> bf16 in one 128-lane op
            w = hpool.tile([128, 256], BF16, tag="wev", name=f"wev_{t}")
            nc.vector.tensor_scalar_mul(w[:], ps[:, 0:256], INV_SW)
            last = (t == S - 1)
            hdr_new = []
            if last:
                hf = []
            for c in range(4):
                pt = psT.tile([128, 32], BF16, tag=f"pt{c}", name=f"pt{c}_{t}",
                              bufs=1)
                for e in range(2):
                    nc.tensor.transpose(pt[:, 16 * e:16 * (e + 1)],
                                        w[32 * c:32 * c + 16, 128 * e:128 * (e + 1)],
                                        eyeb[32 * c:32 * c + 16, :],
                                        tile_position=(32 * c, 0))
                # s = S_H*(h@Wh2.T)^T + S_H*U^T  (chunk c covers j-chunks 2c,2c+1)
                sc = hpool.tile([128, 2, 16], BF16, tag=f"s{c}", name=f"s{c}_{t}")
                nc.vector.tensor_add(sc[:, :, :], pt[:],
                                     ring[:, 2 * c:2 * c + 2,
                                          BL * g:BL * (g + 1)])
                ht = hpool.tile([128, 2, 16], F8, tag=f"hdr{c}", name=f"hdr{c}_{t}")
                nc.scalar.activation(ht[:, :, :], sc[:, :, :], LRELU,
                                     bias=0.0, scale=1.0, alpha=ALPHA)
                hdr_new.append(ht)
                if last:
                    hfc = hpool.tile([128, 2, 16], F32R, tag=f"hf{c}", name=f"hf{c}")
                    nc.scalar.activation(hfc[:, :, :], sc[:, :, :], LRELU,
                                         bias=0.0, scale=1.0, alpha=ALPHA)
                    hf.append(hfc)
            hdr = hdr_new

        # ---- phase 3: out = h_S @ (Wo.T/S_H) + bo  (hf holds S_H*h_S) ----
        po = psR.tile([16, O], F32, tag="psO", name="psO", bufs=1)
        nc.tensor.matmul(po[:], ones[0:1, 0:16], bo2[0:1, :],
                         start=True, stop=False)
        for k in range(8):
            c, e = k // 2, k % 2
            nc.tensor.matmul(po[:], hf[c][:, e, :], wot[k][:],
                             start=False, stop=(k == 7))
        osb = opool.tile([16, O], F32, tag="osb", name="osb")
        nc.vector.tensor_copy(osb[:], po[:])
        nc.sync.dma_start(y_d.ap(), osb[:])

    nc.compile()
    return nc


def _prep_inputs(x, Wi, bi, Wh, bh, Wo, bo):
    W2 = Wh[:, H:]                                  # [H(j), H(k)]
    w2t = np.ascontiguousarray(W2.T) * S_W          # [k, j]
    shared = {
        "wit": np.ascontiguousarray(Wi.T),
        "wh1t": np.ascontiguousarray(Wh[:, :H].T) * S_H,
        "wh2f8": np.ascontiguousarray(w2t).astype(ml_dtypes.float8_e4m3),
        "wot": np.ascontiguousarray(Wo.T) / S_H,
        "bi": np.ascontiguousarray(bi.reshape(H // 128, 128).T),
        "bh": bh.reshape(1, H).copy() * S_H,
        "bo": bo.reshape(1, O).copy(),
        "eyeb": np.tile(np.eye(16, dtype=np.float32), (8, 1)).astype(ml_dtypes.bfloat16),
        "ones": np.ones((1, 512), np.float32),
    }
    in_maps = []
    for c in range(NCORES):
        xc = x[BL * c:BL * (c + 1)]            # [16, S, I]
        xt = np.ascontiguousarray(
            xc.transpose(2, 1, 0).reshape(I, TOK))  # [I, S*16] col = t*16+b
        m = dict(shared)
        m["xt"] = xt
        in_maps.append(m)
    return in_maps


def kernel(x, Wi, bi, Wh, bh, Wo, bo, _trace=False):
    global _CACHED
    x = np.asarray(x, dtype=np.float32)
    if _CACHED is None:
        _CACHED = _build()
    nc = _CACHED
    in_maps = _prep_inputs(np.asarray(x, np.float32), np.asarray(Wi, np.float32),
                           np.asarray(bi, np.float32), np.asarray(Wh, np.float32),
                           np.asarray(bh, np.float32), np.asarray(Wo, np.float32),
                           np.asarray(bo, np.float32))
    res = run_bass_kernel_spmd(nc, in_maps, list(range(NCORES)), trace=_trace)
    out = np.concatenate([res.results[c]["y"] for c in range(NCORES)], axis=0)
    if _trace:
        return out, res
    return out
